# revision 35
# baseline (speedup 1.0000x reference)
"""BiRWKV Trainium2 kernel.

Strategy
--------
The problem is a bidirectional RWKV layer: two independent RWKV recurrences
(one forward in time, one backward) over x:(T=2048, B=8, I=1024) with
per-direction weights (H=1024).  Outputs are concatenated to (T, B, 2H), and
the "hidden" return is just slices of the full output (last fw step / first
bw step), so only the full output needs computing on device.

Sharding: 2 directions x 8 batches = 16 independent sequence units; each of
the 8 cores gets one direction and 2 batches (cores 0-3: forward, batches
2c..2c+1; cores 4-7: backward via host-side time reversal of the input).
Weights are replicated per-direction.  Every core runs the identical program
(SPMD) on different data.

Per-core kernel:
  - r/k/v projections: bf16 matmuls, weights stationary, x^T streamed
    ([i,t] layout, pre-transposed on host), accumulated in PSUM over 8
    k-tiles of 128.
  - r = sigmoid(k_r) on ScalarE straight out of PSUM; ek = exp(k);
    euk = exp(k + u) using the per-partition activation bias.
  - The RWKV recurrence is computed UNSTABILIZED (mathematically identical;
    with this problem's value ranges (|k| <~ 4, w ~ -1) fp32 never
    overflows):  N_t = e^w N_{t-1} + ek_t v_t ; D_t = e^w D_{t-1} + ek_t ;
    y_t = r_t (N_{t-1} + e^u ek_t v_t) / (D_{t-1} + e^u ek_t).
    N and D are produced by the DVE `tensor_tensor_scan` instruction
    (state = data0*state + data1 along the free axis) with channel h on
    partitions and time on the free axis - no per-timestep loop at all.
  - 1/denom is exp(-ln(denom)) on ScalarE (keeps the division off the DVE,
    which is the busiest engine).
  - Time is processed in blocks of 512 with the scan state chained through
    a carry column; (direction-batch, h-tile) units are independent, so the
    Tile scheduler overlaps PE matmuls of one unit with DVE scan work of
    the previous one.
"""

import numpy as np
import ml_dtypes

T, B, I, H = 2048, 8, 1024, 1024
NCORES = 8
BL = 2               # batches per core
TB = 512             # time block (free dim per instruction)
NBLK = T // TB
KT = I // 128        # k tiles (contraction)
MT = H // 128        # m tiles (output channels)

_CACHE = {}


def _build(t_len=T, tb=TB, y_f32=True, sliced_dma=True, safe_prec=True):
    from contextlib import ExitStack
    import concourse.bass as bass
    import concourse.tile as tile
    from concourse import bacc, mybir

    f32 = mybir.dt.float32
    bf16 = mybir.dt.bfloat16
    AF = mybir.ActivationFunctionType
    OP = mybir.AluOpType

    nblk = t_len // tb

    nc = bacc.Bacc("TRN2", target_bir_lowering=False, debug=False)
    xT = nc.dram_tensor("xT", [BL, KT, 128, t_len], bf16, kind="ExternalInput").ap()
    wrT = nc.dram_tensor("wrT", [KT, 128, H], bf16, kind="ExternalInput").ap()
    wkT = nc.dram_tensor("wkT", [KT, 128, H], bf16, kind="ExternalInput").ap()
    wvT = nc.dram_tensor("wvT", [KT, 128, H], bf16, kind="ExternalInput").ap()
    uvec = nc.dram_tensor("uvec", [128, MT], f32, kind="ExternalInput").ap()
    euvec = nc.dram_tensor("euvec", [128, MT], f32, kind="ExternalInput").ap()
    ewbd = nc.dram_tensor("ewbd", [128, MT * tb], f32, kind="ExternalInput").ap()
    y_dt = f32 if y_f32 else bf16
    # sc = dtype of the scan-state / combine chain. bf16 inputs (matmul, exp
    # outputs) stay bf16 either way; safe_prec keeps the accumulation and
    # output path fp32, trading DVE cycles (which have slack under PE) for
    # ~2x lower output error.
    sc = f32 if safe_prec else bf16
    y = nc.dram_tensor("y", [BL, MT, 128, t_len], y_dt, kind="ExternalOutput").ap()

    with tile.TileContext(nc) as tc, ExitStack() as ctx:
        consts = ctx.enter_context(tc.tile_pool(name="consts", bufs=1))
        wts = ctx.enter_context(tc.tile_pool(name="wts", bufs=1))
        xts = ctx.enter_context(tc.tile_pool(name="xts", bufs=1))
        work = ctx.enter_context(tc.tile_pool(name="work", bufs=2))
        psum = ctx.enter_context(tc.tile_pool(name="psum", bufs=2, space="PSUM"))

        # DMA issue order matters: the (modeled) HWDGE pipe drains in order,
        # so load exactly what the first matmul group needs first: wr tiles,
        # then x block j=0, then the rest.
        wtiles = {}
        for name, dram in (("r", wrT), ("k", wkT), ("v", wvT)):
            for k in range(KT):
                t = wts.tile([128, H], bf16, tag=f"w{name}{k}", name=f"w{name}{k}")
                wtiles[name, k] = t

        for k in range(KT):
            nc.sync.dma_start(wtiles["r", k][:], wrT[k])

        all_xtiles = {}
        for b in range(BL):
            for k in range(KT):
                t = xts.tile([128, t_len], bf16, tag=f"xt{k}", name=f"xt{b}_{k}",
                             bufs=2)
                all_xtiles[b, k] = t
        if sliced_dma:
            for k in range(KT):
                nc.sync.dma_start(all_xtiles[0, k][:, 0:tb], xT[0, k, :, 0:tb])

        for k in range(KT):
            nc.sync.dma_start(wtiles["k", k][:], wkT[k])
        for k in range(KT):
            nc.sync.dma_start(wtiles["v", k][:], wvT[k])

        u_t = consts.tile([128, MT], f32, tag="u")
        nc.sync.dma_start(u_t[:], uvec)
        eu_t = consts.tile([128, MT], f32, tag="eu")
        nc.sync.dma_start(eu_t[:], euvec)
        # e^w broadcast along the free (time) axis, one [128, tb] strip per
        # h-tile, used as data0 of the scans (precomputed on host).
        ewb = consts.tile([128, MT * tb], f32, tag="ewb")
        nc.sync.dma_start(ewb[:], ewbd)

        if sliced_dma:
            for b in range(BL):
                for j in range(nblk):
                    if b == 0 and j == 0:
                        continue
                    for k in range(KT):
                        nc.sync.dma_start(
                            all_xtiles[b, k][:, j * tb:(j + 1) * tb],
                            xT[b, k, :, j * tb:(j + 1) * tb])
        else:
            for b in range(BL):
                for k in range(KT):
                    nc.sync.dma_start(all_xtiles[b, k][:], xT[b, k])

        for b in range(BL):
            xtiles = [all_xtiles[b, k] for k in range(KT)]
            # j outer / m inner: block j=0 over all h-tiles only needs the
            # first x slice + weights, so the DMA prologue hides behind PE
            # work; the scan carry chains (per m) get 8 iterations of slack.
            n_prev_m = {}
            d_prev_m = {}
            for j in range(nblk):
                for m in range(MT):
                    n_prev = n_prev_m.get(m)
                    d_prev = d_prev_m.get(m)
                    tsl = slice(j * tb, (j + 1) * tb)
                    psr = psum.tile([128, tb], f32, tag="psr", name="psr", bufs=2)
                    psk = psum.tile([128, tb], f32, tag="psk", name="psk", bufs=2)
                    psv = psum.tile([128, tb], f32, tag="psv", name="psv", bufs=2)
                    for ps, wname in ((psr, "r"), (psk, "k"), (psv, "v")):
                        for k in range(KT):
                            nc.tensor.matmul(
                                ps[:],
                                wtiles[wname, k][:, m * 128:(m + 1) * 128],
                                xtiles[k][:, tsl],
                                start=(k == 0),
                                stop=(k == KT - 1),
                            )
                    # er = e^{-k_r}; the output gate sigmoid(k_r) is folded
                    # into the denominator: y = numer / (denom * (1 + er)).
                    # Elementwise chain runs in bf16 for the DVE 2x/4x modes;
                    # the scan state feedback stays fp32 inside the HW.
                    er = work.tile([128, tb], bf16, tag="er", name="er")
                    ek = work.tile([128, tb], bf16, tag="ek", name="ek")
                    euk = work.tile([128, tb], bf16, tag="euk", name="euk")
                    v_sb = work.tile([128, tb], bf16, tag="v_sb", name="v_sb")
                    ekv = work.tile([128, tb], bf16, tag="ekv", name="ekv")
                    eukv = work.tile([128, tb], bf16, tag="eukv", name="eukv")
                    # Scan state buffers: col 0 = carry (state before this
                    # block), cols 1..tb = inclusive scan outputs. bufs must
                    # cover a full m-sweep so block j-1's state is still
                    # alive when block j of the same m reads it.
                    nb = work.tile([128, tb + 1], sc, tag="Nb", name="Nb",
                                   bufs=MT + 1)
                    db = work.tile([128, tb + 1], sc, tag="Db", name="Db",
                                   bufs=MT + 1)
                    numer = work.tile([128, tb], sc, tag="numer", name="numer")
                    denom = work.tile([128, tb], sc, tag="denom", name="denom")
                    t0 = work.tile([128, tb], sc, tag="t0", name="t0")
                    dd = work.tile([128, tb], sc, tag="dd", name="dd")
                    # ldd must stay fp32: bf16 absolute error on ln blows up
                    # through exp(-ldd).
                    ldd = work.tile([128, tb], f32, tag="ldd", name="ldd")
                    rdd = work.tile([128, tb], sc, tag="rdd", name="rdd")
                    yt = work.tile([128, tb], y_dt, tag="yt", name="yt")

                    if j == 0:
                        nc.vector.memset(nb[:, 0:1], 0.0)
                        nc.vector.memset(db[:, 0:1], 0.0)
                    else:
                        nc.vector.tensor_copy(nb[:, 0:1], n_prev[:, tb:tb + 1])
                        nc.vector.tensor_copy(db[:, 0:1], d_prev[:, tb:tb + 1])

                    def chunk(lo, hi):
                        s = slice(lo, hi)
                        nc.scalar.activation(er[:, s], psr[:, s], AF.Exp,
                                             scale=-1.0)
                        nc.scalar.activation(ek[:, s], psk[:, s], AF.Exp)
                        nc.scalar.activation(euk[:, s], psk[:, s], AF.Exp,
                                             bias=u_t[:, m:m + 1])
                        nc.scalar.copy(v_sb[:, s], psv[:, s])
                        nc.vector.tensor_tensor(ekv[:, s], ek[:, s], v_sb[:, s],
                                                OP.mult)
                        nc.vector.tensor_scalar(
                            eukv[:, s], ekv[:, s], eu_t[:, m:m + 1], None,
                            OP.mult)
                        ewm = ewb[:, m * tb + lo:m * tb + hi]
                        nc.vector.tensor_tensor_scan(
                            nb[:, lo + 1:hi + 1], ewm, ekv[:, s],
                            nb[:, lo:lo + 1], OP.mult, OP.add)
                        nc.vector.tensor_tensor_scan(
                            db[:, lo + 1:hi + 1], ewm, ek[:, s],
                            db[:, lo:lo + 1], OP.mult, OP.add)
                        nc.vector.tensor_tensor(numer[:, s], nb[:, lo:hi],
                                                eukv[:, s], OP.add)
                        nc.vector.tensor_tensor(denom[:, s], db[:, lo:hi],
                                                euk[:, s], OP.add)
                        # dd = denom * (1 + er);  y = numer * exp(-ln(dd))
                        nc.vector.tensor_scalar(t0[:, s], er[:, s], 1.0, None,
                                                OP.add)
                        nc.vector.tensor_tensor(dd[:, s], denom[:, s], t0[:, s],
                                                OP.mult)
                        nc.scalar.activation(ldd[:, s], dd[:, s], AF.Ln)
                        nc.scalar.activation(rdd[:, s], ldd[:, s], AF.Exp,
                                             scale=-1.0)
                        nc.vector.tensor_tensor(yt[:, s], numer[:, s],
                                                rdd[:, s], OP.mult)
                        nc.sync.dma_start(y[b, m, :, j * tb + lo:j * tb + hi],
                                          yt[:, s])

                    chunk(0, tb)
                    n_prev_m[m] = nb
                    d_prev_m[m] = db
    # All our activations are Exp and Ln; steer the table chooser to the one
    # set containing both so no table reloads happen inside the main loop.
    # The dict's insertion order IS the act_func_set_id, so keep every entry
    # and only remove Exp/Ln from the other sets.
    import concourse.bacc as bacc_mod
    orig = bacc_mod.get_activation_tables

    def pinned(arch):
        tabs = orig(arch)
        strip = {mybir.ActivationFunctionType.Exp, mybir.ActivationFunctionType.Ln}
        return {
            k: (v if k == "natural_log_exp_and_others" else v - strip)
            for k, v in tabs.items()
        }

    bacc_mod.get_activation_tables = pinned
    try:
        nc.compile()
    finally:
        bacc_mod.get_activation_tables = orig
    return nc


def _get_nc():
    if "nc" not in _CACHE:
        _CACHE["nc"] = _build()
    return _CACHE["nc"]


def _pack_x(xc):
    # xc: (T, BL, I) float32 -> (BL, KT, 128, T) bf16
    xt = np.ascontiguousarray(xc.transpose(1, 2, 0)).reshape(BL, KT, 128, xc.shape[0])
    return xt.astype(ml_dtypes.bfloat16)


def _pack_w(w):
    # w: (H, I) -> W^T (I, H) -> (KT, 128, H) bf16
    return np.ascontiguousarray(w.T).reshape(KT, 128, H).astype(ml_dtypes.bfloat16)


def _pack_vec(v):
    # v: (H,) -> (128, MT) f32 with [p, m] = v[m*128 + p]
    return np.ascontiguousarray(v.reshape(MT, 128).T).astype(np.float32)


def kernel(**inputs):
    from concourse.bass_utils import run_bass_kernel_spmd

    x = np.asarray(inputs["inputs"], dtype=np.float32)
    packs = {}
    for d in ("fw", "bw"):
        packs[d] = {
            "wrT": _pack_w(np.asarray(inputs[f"w_r_{d}"], dtype=np.float32)),
            "wkT": _pack_w(np.asarray(inputs[f"w_k_{d}"], dtype=np.float32)),
            "wvT": _pack_w(np.asarray(inputs[f"w_v_{d}"], dtype=np.float32)),
            "uvec": _pack_vec(np.asarray(inputs[f"w_u_{d}"], dtype=np.float32)),
            "euvec": _pack_vec(
                np.exp(np.asarray(inputs[f"w_u_{d}"], dtype=np.float32))),
            "ewbd": np.ascontiguousarray(np.repeat(
                _pack_vec(np.exp(np.asarray(inputs[f"w_w_{d}"], dtype=np.float32))),
                TB, axis=1)),
        }

    in_maps = []
    for c in range(NCORES):
        if c < 4:
            xc = x[:, 2 * c:2 * c + 2, :]
            d = "fw"
        else:
            xc = x[::-1, 2 * (c - 4):2 * (c - 4) + 2, :]
            d = "bw"
        in_maps.append({"xT": _pack_x(xc), **packs[d]})

    nc = _get_nc()
    res = run_bass_kernel_spmd(nc, in_maps, core_ids=list(range(NCORES)))
    _CACHE["last_res"] = res

    output = np.empty((T, B, 2 * H), dtype=np.float32)
    for c in range(NCORES):
        yc = res.results[c]["y"]  # (BL, MT, 128, T) bf16
        yc = np.ascontiguousarray(
            yc.transpose(3, 0, 1, 2)).reshape(T, BL, H).astype(np.float32)
        if c < 4:
            output[:, 2 * c:2 * c + 2, :H] = yc
        else:
            output[:, 2 * (c - 4):2 * (c - 4) + 2, H:] = yc[::-1]

    hidden = np.stack([output[-1, :, :H], output[0, :, H:]], axis=0)
    return output, hidden
